# revision 21
# baseline (speedup 1.0000x reference)
"""Trainium2 Bass kernel for tiny-sequence causal attention.

Problem: x [B=131072, P=3, D=128], H=4 heads x DH=32. Causal attention over
P=3 positions, then output projection. Data-parallel over 8 NeuronCores
(batch sharded); weights replicated.

Layout strategy ("transposed world"): on-chip, features live on the 128
partitions and tokens stream along the free dimension. Then all four
projections are plain PE matmuls with stationary weights, and the per-head
score reduction (sum over DH=32 within a head) is a single PE matmul with a
block-diagonal ones matrix J that simultaneously broadcasts each head's
score to all 32 of its partitions. The only elementwise work (DVE/ACT) is
the 3x3 causal softmax and the position-mix of V, all batched along the
free dim.

Causal softmax for P=3:
  row q=0: prob = [1]                    -> z0 = v0 (free)
  row q=1: 2-way softmax == sigmoid      -> no reciprocal needed
  row q=2: 3-way softmax, shifted by s22 -> one reciprocal of (1+e20'+e21')
"""

import numpy as np

B, P, D = 131072, 3, 128
H, DH = 4, 32
F = H * DH  # 128
NCORES = 8
BC = B // NCORES  # 16384 batches per core
TOK = BC * P      # 49152 tokens per core
GB = 128          # batches per group
GT = GB * P       # 384 tokens per group
NG = BC // GB     # 128 groups
INVS = 1.0 / float(np.sqrt(DH))

_CACHE = {}


def _split_multiwaits(nc, mybir):
    """walrus in this toolchain accepts at most ONE sync-wait per
    instruction. Split any instruction carrying k>1 waits into k-1
    preceding single-wait NoOps on the same engine (same queue order, same
    semaphore semantics) plus the original instruction with the last wait."""
    cnt = 0
    for name, bbb in nc.bb_map.items():
        insts = bbb.bb.instructions
        if not insts:
            continue
        out = []
        changed = False
        for inst in insts:
            si = inst.sync_info
            if si is not None and si.on_wait and len(si.on_wait) > 1:
                waits = list(si.on_wait)
                for w in waits[:-1]:
                    nop = mybir.InstNoOp(name=f"wsplit_{cnt}", ins=[], outs=[])
                    cnt += 1
                    nop.engine = inst.engine
                    nop.sync_info = mybir.SyncInfo(on_wait=[w], on_update=[])
                    out.append(nop)
                inst.sync_info = mybir.SyncInfo(
                    on_wait=[waits[-1]], on_update=list(si.on_update or [])
                )
                changed = True
            out.append(inst)
        if changed:
            bbb.bb.instructions[:] = out
    return cnt


def _build_nc():
    import concourse.bass as bass
    import concourse.mybir as mybir
    from concourse.tile import TileContext
    from concourse import masks

    f32 = mybir.dt.float32
    f32r = mybir.dt.float32r
    AF = mybir.ActivationFunctionType
    ALU = mybir.AluOpType

    nc = bass.Bass()
    x_d = nc.declare_dram_parameter("x", [TOK, D], f32, isOutput=False)
    wq_d = nc.declare_dram_parameter("wq", [D, F], f32, isOutput=False)
    wk_d = nc.declare_dram_parameter("wk", [D, F], f32, isOutput=False)
    wv_d = nc.declare_dram_parameter("wv", [D, F], f32, isOutput=False)
    wo_d = nc.declare_dram_parameter("wo", [F, D], f32, isOutput=False)
    jm_d = nc.declare_dram_parameter("jm", [F, F], f32, isOutput=False)
    out_d = nc.declare_dram_parameter("out", [TOK, D], f32, isOutput=True)

    with TileContext(nc) as tc:
        with (
            tc.tile_pool(name="wpool", bufs=1) as wpool,
            tc.tile_pool(name="work", bufs=3) as wp,
            tc.tile_pool(name="ps_big", bufs=2, space="PSUM") as ps_big_pool,
            tc.tile_pool(name="ps_q", bufs=1, space="PSUM") as ps_q_pool,
            tc.tile_pool(name="ps_k", bufs=1, space="PSUM") as ps_k_pool,
            tc.tile_pool(name="ps_v", bufs=1, space="PSUM") as ps_v_pool,
            tc.tile_pool(name="ps_s1", bufs=1, space="PSUM") as ps_s1_pool,
            tc.tile_pool(name="ps_s2", bufs=1, space="PSUM") as ps_s2_pool,
            tc.tile_pool(name="ps_warm", bufs=1, space="PSUM") as ps_warm_pool,
        ):
            # Matmult instructions (self-loading fp32 / transpose) have a
            # single sync-wait slot, so every operand a PE instruction might
            # freshly wait on is staged through ACT: the PE then only ever
            # needs one wait (on ACT) the first time, and Tile's wait elision
            # covers the rest via monotone per-processor clocks.
            ident_st = wpool.tile([128, 128], f32)
            masks.make_identity(nc, ident_st[:])
            ident = wpool.tile([128, 128], f32)
            nc.scalar.copy(ident[:], ident_st[:])
            w_stage = {}
            w_sb = {}
            for nm, dram in (
                ("wq", wq_d), ("wk", wk_d), ("wv", wv_d), ("wo", wo_d), ("jm", jm_d)
            ):
                st = wpool.tile([128, 128], f32, tag=f"st_{nm}")
                nc.sync.dma_start(st[:], dram[:])
                sb = wpool.tile([128, 128], f32r, tag=f"sb_{nm}")
                nc.scalar.copy(sb[:], st[:])
                w_stage[nm] = st
                w_sb[nm] = sb
            wq_s, wk_s, wv_s = w_sb["wq"], w_sb["wk"], w_sb["wv"]
            wo_s, jm_s = w_sb["wo"], w_sb["jm"]

            for g in range(NG):
                t0 = g * GT
                # PE "clock anchor": a dummy transpose whose PSUM-slot WAW
                # forces one PE self-wait covering all of the previous
                # group's PE ticks, so every real matmul's same-bank WAW is
                # elided and its single wait slot stays free for its real
                # cross-engine dependency.
                warm = ps_warm_pool.tile([128, 128], f32, tag="warm")
                nc.tensor.transpose(warm[:], ident[:], ident[:])
                # ---- load x (partition=token%128, free=(j,d)) ----
                xr = wp.tile([128, P, D], f32, tag="xr")
                if g == 0:
                    # Group 0's transposes must not need waits on both the
                    # DMA queue and ACT (identity) -> stage through ACT once.
                    xr_st = wp.tile([128, P, D], f32, tag="xr_st")
                    nc.sync.dma_start(
                        xr_st[:],
                        x_d[t0 : t0 + GT, :].rearrange("(j p) d -> p j d", p=128),
                    )
                    nc.scalar.copy(xr[:], xr_st[:])
                else:
                    nc.sync.dma_start(
                        xr[:],
                        x_d[t0 : t0 + GT, :].rearrange("(j p) d -> p j d", p=128),
                    )
                # ---- transpose to [d, token] ----
                xt_ps = ps_big_pool.tile([128, GT], f32, tag="big")
                for j in range(P):
                    nc.tensor.transpose(
                        xt_ps[:, j * 128 : (j + 1) * 128], xr[:, j, :], ident[:]
                    )
                xt = wp.tile([128, GT], f32r, tag="xt")
                nc.scalar.copy(xt[:], xt_ps[:])

                # ---- QKV projections: [f, token] in PSUM ----
                ps_q = ps_q_pool.tile([F, GT], f32, tag="ps_q")
                ps_k = ps_k_pool.tile([F, GT], f32, tag="ps_k")
                ps_v = ps_v_pool.tile([F, GT], f32, tag="ps_v")
                nc.tensor.matmul(ps_q[:], wq_s[:], xt[:], start=True, stop=True)
                nc.tensor.matmul(ps_k[:], wk_s[:], xt[:], start=True, stop=True)
                nc.tensor.matmul(ps_v[:], wv_s[:], xt[:], start=True, stop=True)
                # Stage only Q rows 1,2 in SBUF; K and V are read directly
                # from PSUM (TT ops allow one PSUM operand).
                q12 = wp.tile([128, 2, GB], f32, tag="q12")
                nc.scalar.copy(
                    q12[:], ps_q[:].rearrange("f (b t) -> f t b", t=P)[:, 1:3, :]
                )
                kv = ps_k[:].rearrange("f (b t) -> f t b", t=P)  # [f, pos, batch]
                vv = ps_v[:].rearrange("f (b t) -> f t b", t=P)

                # ---- score element-products (5 causal pairs) ----
                e = wp.tile([128, 5, GB], f32r, tag="e")
                nc.vector.tensor_mul(e[:, 0, :], q12[:, 0, :], kv[:, 0, :])
                nc.vector.tensor_mul(e[:, 1, :], q12[:, 0, :], kv[:, 1, :])
                nc.vector.tensor_mul(e[:, 2, :], q12[:, 1, :], kv[:, 0, :])
                nc.vector.tensor_mul(e[:, 3, :], q12[:, 1, :], kv[:, 1, :])
                nc.vector.tensor_mul(e[:, 4, :], q12[:, 1, :], kv[:, 2, :])

                # ---- per-head sums (+ broadcast to all 32 lanes of head) ----
                s1_ps = ps_s1_pool.tile([128, 2 * GB], f32, tag="s1_ps")
                s2_ps = ps_s2_pool.tile([128, 3 * GB], f32, tag="s2_ps")
                nc.tensor.matmul(
                    s1_ps[:], jm_s[:], e[:, 0:2, :],
                    start=True, stop=True,
                )
                nc.tensor.matmul(
                    s2_ps[:], jm_s[:], e[:, 2:5, :],
                    start=True, stop=True,
                )
                s2v = s2_ps[:].rearrange("f (j b) -> f j b", j=3)
                s11s = wp.tile([128, GB], f32, tag="s11s")
                nc.scalar.copy(s11s[:], s1_ps[:, GB : 2 * GB])
                s22s = wp.tile([128, GB], f32, tag="s22s")
                nc.scalar.copy(s22s[:], s2v[:, 2, :])

                # ---- softmax row q=1 via sigmoid ----
                d1 = wp.tile([128, GB], f32, tag="d1")
                nc.vector.tensor_sub(d1[:], s1_ps[:, 0:GB], s11s[:])
                p10 = wp.tile([128, GB], f32, tag="p10")
                p11 = wp.tile([128, GB], f32, tag="p11")
                nc.scalar.activation(p10[:], d1[:], AF.Sigmoid, scale=INVS)
                nc.scalar.activation(p11[:], d1[:], AF.Sigmoid, scale=-INVS)

                # ---- softmax row q=2 (shift by s22) ----
                d2 = wp.tile([128, 2, GB], f32, tag="d2")
                nc.vector.tensor_sub(d2[:, 0, :], s2v[:, 0, :], s22s[:])
                nc.vector.tensor_sub(d2[:, 1, :], s2v[:, 1, :], s22s[:])
                e2 = wp.tile([128, 2, GB], f32, tag="e2")
                nc.scalar.activation(e2[:], d2[:], AF.Exp, scale=INVS)
                t2b = wp.tile([128, GB], f32, tag="t2b")
                nc.vector.scalar_tensor_tensor(
                    t2b[:], e2[:, 0, :], 1.0, e2[:, 1, :],
                    op0=ALU.add, op1=ALU.add,
                )  # (e20'+1) + e21'
                p2 = wp.tile([128, 3, GB], f32, tag="p2")
                nc.vector.reciprocal(p2[:, 2, :], t2b[:])
                nc.vector.tensor_mul(p2[:, 0, :], e2[:, 0, :], p2[:, 2, :])
                nc.vector.tensor_mul(p2[:, 1, :], e2[:, 1, :], p2[:, 2, :])

                # ---- z mix (token-interleaved [f, token]) ----
                zi = wp.tile([128, GT], f32r, tag="zi")
                ziv = zi[:].rearrange("f (b t) -> f t b", t=P)
                nc.vector.tensor_copy(ziv[:, 0, :], vv[:, 0, :])  # z0 = v0
                t1a = wp.tile([128, GB], f32, tag="t1a")
                t1b = wp.tile([128, GB], f32, tag="t1b")
                nc.vector.tensor_mul(t1a[:], p10[:], vv[:, 0, :])
                nc.vector.tensor_mul(t1b[:], p11[:], vv[:, 1, :])
                nc.vector.tensor_add(ziv[:, 1, :], t1a[:], t1b[:])
                zp = wp.tile([128, GB, 3], f32, tag="zp")
                nc.vector.tensor_mul(
                    zp[:], p2[:].transpose([0, 2, 1]), vv[:].transpose([0, 2, 1])
                )
                with nc.allow_low_precision(reason="3-term reduce; f32r out"):
                    nc.vector.tensor_reduce(
                        ziv[:, 2, :], zp[:], axis=mybir.AxisListType.X, op=ALU.add
                    )

                # ---- O projection ----
                ot_ps = ps_big_pool.tile([D, GT], f32, tag="big")
                nc.tensor.matmul(ot_ps[:], wo_s[:], zi[:], start=True, stop=True)
                ots = wp.tile([D, GT], f32, tag="ots")
                nc.scalar.copy(ots[:], ot_ps[:])

                # ---- transpose back to [token, d] and store ----
                out_ps = ps_big_pool.tile([128, GT], f32, tag="big")
                outs = wp.tile([128, P, D], f32, tag="outs")
                for j in range(P):
                    nc.tensor.transpose(
                        out_ps[:, j * 128 : (j + 1) * 128],
                        ots[:, j * 128 : (j + 1) * 128],
                        ident[:],
                    )
                nc.vector.tensor_copy(outs[:], out_ps[:])
                nc.sync.dma_start(
                    out_d[t0 : t0 + GT, :].rearrange("(j p) d -> p j d", p=128),
                    outs[:],
                )
                warm2 = ps_warm_pool.tile([128, 128], f32, tag="warm")
                nc.tensor.transpose(warm2[:], ident[:], ident[:])
    n = _split_multiwaits(nc, mybir)
    return nc


def _prep_weights(W_Q, W_K, W_V, W_O):
    wq_l = np.ascontiguousarray(W_Q.reshape(F, D).T, dtype=np.float32)
    wk_l = np.ascontiguousarray(W_K.reshape(F, D).T, dtype=np.float32)
    wv_l = np.ascontiguousarray(W_V.reshape(F, D).T, dtype=np.float32)
    wo_l = np.ascontiguousarray(W_O.T, dtype=np.float32)
    jm = np.kron(np.eye(H, dtype=np.float32), np.ones((DH, DH), dtype=np.float32))
    jm = np.ascontiguousarray(jm, dtype=np.float32)
    return wq_l, wk_l, wv_l, wo_l, jm


def _run(x, W_Q, W_K, W_V, W_O, trace=False):
    from concourse.bass_utils import run_bass_kernel_spmd

    if "nc" not in _CACHE:
        _CACHE["nc"] = _build_nc()
    nc = _CACHE["nc"]
    wq_l, wk_l, wv_l, wo_l, jm = _prep_weights(
        np.asarray(W_Q, dtype=np.float32),
        np.asarray(W_K, dtype=np.float32),
        np.asarray(W_V, dtype=np.float32),
        np.asarray(W_O, dtype=np.float32),
    )
    x = np.asarray(x, dtype=np.float32)
    in_maps = []
    for c in range(NCORES):
        xs = np.ascontiguousarray(
            x[c * BC : (c + 1) * BC].reshape(TOK, D), dtype=np.float32
        )
        in_maps.append(
            {"x": xs, "wq": wq_l, "wk": wk_l, "wv": wv_l, "wo": wo_l, "jm": jm}
        )
    res = run_bass_kernel_spmd(nc, in_maps, list(range(NCORES)), trace=trace)
    parts = [np.asarray(res.results[c]["out"]).reshape(BC, P, D) for c in range(NCORES)]
    out = np.concatenate(parts, axis=0)
    return out, res


def kernel(x, W_Q, W_K, W_V, W_O):
    out, _ = _run(x, W_Q, W_K, W_V, W_O, trace=False)
    return out
